# revision 23
# baseline (speedup 1.0000x reference)
"""GCN block (GCNConv + LayerNorm + ReLU) on 8 Trainium2 NeuronCores.

v3: like v2 (precomputed fp8 one-hot S stream, dinv-prescaled gather
tables, identity-rhs self-loops, fused bias/LN/ReLU epilogue) but the
message streams are PACKED: within a segment of SEG_BLOCKS dst blocks,
each core's messages are laid out back-to-back with NO per-block tile
padding (padding only at segment ends, to the max core's tile count).
Block -> tile mapping becomes data-dependent, so each block's PSUM
accumulation covers the UNION across cores of the tiles its messages can
land in (a static interval, from the per-core cumulative counts); the
per-core S tiles zero out the slots that belong to other blocks.
"""

import math
import sys

sys.path.insert(0, "/opt/trn_rl_repo")

import numpy as np
import ml_dtypes

N_NODES = 50000
WIDTH = 256
N_CORES = 8
NODES_PER_CORE = N_NODES // N_CORES  # 6250
P = 128
N_BLOCKS = math.ceil(NODES_PER_CORE / P)  # 49 (last block has 106 rows)
LN_EPS = 1e-5
HALF = N_NODES // 2  # rows per gather table
SEG_BLOCKS = 4  # dst blocks packed per gather segment
GATHER_TILE_CAP = 8  # max tiles (128 idxs) per dma_gather call (HW limit 1024)


def _preprocess(edge_index):
    """Pack non-self-loop messages per (core, segment, parity) contiguously.

    Returns (meta, deg, idxe, idxo, scon) where meta carries the static
    structure shared by all cores:
      meta = (segtiles_e, segtiles_o,   # [n_segs] tiles per segment/parity
              lo_e, hi_e, lo_o, hi_o,   # [N_BLOCKS] participation intervals
                                        # (tile idx relative to segment start)
              soff)                     # [N_BLOCKS] S-stream tile offset
    """
    src = np.asarray(edge_index[0]).astype(np.int64)
    dst = np.asarray(edge_index[1]).astype(np.int64)

    deg = (np.bincount(dst, minlength=N_NODES) + 1).astype(np.float64)  # + self loop

    core = dst // NODES_PER_CORE
    r = dst % NODES_PER_CORE
    blk = np.minimum(r // P, N_BLOCKS - 1)
    dcol = r - blk * P
    tab = src & 1
    gbin = (core * N_BLOCKS + blk) * 2 + tab

    order = np.argsort(gbin, kind="stable")
    src, dcol, gbin = src[order], dcol[order], gbin[order]
    c = gbin // (N_BLOCKS * 2)
    b = (gbin // 2) % N_BLOCKS
    t = gbin & 1

    cnt = np.bincount(gbin, minlength=N_CORES * N_BLOCKS * 2).reshape(
        N_CORES, N_BLOCKS, 2
    )
    n_segs = math.ceil(N_BLOCKS / SEG_BLOCKS)
    seg_of = np.arange(N_BLOCKS) // SEG_BLOCKS

    # per-core cumulative counts within each segment -> slot of each message
    # start[c, b, t] = within-segment start slot of block b's bucket
    start = np.zeros((N_CORES, N_BLOCKS, 2), np.int64)
    for s in range(n_segs):
        bs = np.where(seg_of == s)[0]
        cum = np.cumsum(cnt[:, bs, :], axis=1)
        start[:, bs[1:], :] = cum[:, :-1, :]
    end = start + cnt  # within-segment end slot

    segtiles = np.zeros((n_segs, 2), np.int64)
    for s in range(n_segs):
        bs = np.where(seg_of == s)[0]
        tot = cnt[:, bs, :].sum(axis=1)  # [8, 2]
        segtiles[s] = np.ceil(tot.max(axis=0) / P).astype(np.int64)
    segtiles_e = segtiles[:, 0]
    segtiles_o = segtiles[:, 1]

    # participation intervals (tiles relative to segment start)
    lo = np.zeros((N_BLOCKS, 2), np.int64)
    hi = np.zeros((N_BLOCKS, 2), np.int64)
    for bb in range(N_BLOCKS):
        s = seg_of[bb]
        for tt in range(2):
            lo[bb, tt] = start[:, bb, tt].min() // P
            hi[bb, tt] = min(
                int(np.ceil(end[:, bb, tt].max() / P)), int(segtiles[s, tt])
            )
            hi[bb, tt] = max(hi[bb, tt], lo[bb, tt])  # empty-bucket guard
    we = (hi - lo)[:, 0]
    wo = (hi - lo)[:, 1]
    soff = np.concatenate([[0], np.cumsum(we + wo)])  # S tile offset per block
    s_tiles_tot = int(soff[-1])

    # gather-stream tile offsets per segment
    egoff = np.concatenate([[0], np.cumsum(segtiles_e)])
    ogoff = np.concatenate([[0], np.cumsum(segtiles_o)])
    sTL, sTH = int(egoff[-1]), int(ogoff[-1])

    # per-message placement
    starts_flat = start[c, b, t]  # within-seg start slot of this bucket
    jj = np.zeros(len(gbin), np.int64)
    bstarts = np.concatenate([[0], np.cumsum(cnt.ravel())])[:-1]
    jj = np.arange(len(gbin)) - bstarts[gbin]  # index within bucket
    slot = starts_flat + jj  # within-segment slot
    tile_in_seg = slot // P
    p = slot % P

    idxe_flat = np.zeros((N_CORES, sTL * P), np.int16)
    idxo_flat = np.zeros((N_CORES, sTH * P), np.int16)
    scon = np.zeros((N_CORES, P, s_tiles_tot * P), ml_dtypes.float8_e4m3)

    idx16 = (src >> 1).astype(np.int16)
    seg_m = seg_of[b]
    ev = t == 0
    Je = (egoff[seg_m] + tile_in_seg) * P + p
    Jo = (ogoff[seg_m] + tile_in_seg) * P + p
    idxe_flat[c[ev], Je[ev]] = idx16[ev]
    idxo_flat[c[~ev], Jo[~ev]] = idx16[~ev]

    # per-(core, seg, parity) real counts; tail pads become -1 so a
    # register-supplied num_idxs lets the Q7 skip generating them
    segreal = np.zeros((N_CORES, n_segs, 2), np.int64)
    for s in range(n_segs):
        bs = np.where(seg_of == s)[0]
        segreal[:, s, :] = cnt[:, bs, :].sum(axis=1)
    for cc in range(N_CORES):
        for s in range(n_segs):
            r0 = int(segreal[cc, s, 0])
            idxe_flat[cc, int(egoff[s]) * P + r0 : int(egoff[s + 1]) * P] = -1
            r1 = int(segreal[cc, s, 1])
            idxo_flat[cc, int(ogoff[s]) * P + r1 : int(ogoff[s + 1]) * P] = -1

    # static call plan: CAP-sized windows; a window entirely below every
    # core's real count gets a static count, the rest read a register
    calls = []
    cnts_cols = 0
    for s in range(n_segs):
        for par, tiles in ((0, int(segtiles_e[s])), (1, int(segtiles_o[s]))):
            full = int(segreal[:, s, par].min()) // P
            c0 = 0
            while c0 < tiles:
                c1 = min(c0 + GATHER_TILE_CAP, tiles)
                if c1 <= full:
                    calls.append((s, par, c0, c1, -1))
                else:
                    calls.append((s, par, c0, c1, cnts_cols))
                    cnts_cols += 1
                c0 = c1
    cnts = np.zeros((N_CORES, 1, max(1, cnts_cols)), np.int32)
    for s, par, c0, c1, ridx in calls:
        if ridx >= 0:
            v = np.clip(segreal[:, s, par] - c0 * P, 1, (c1 - c0) * P)
            cnts[:, 0, ridx] = v.astype(np.int32)

    # S stream: per block, even participation tiles then odd
    spos = np.where(
        ev,
        soff[b] + (tile_in_seg - lo[b, 0]),
        soff[b] + we[b] + (tile_in_seg - lo[b, 1]),
    )
    scon[c, p, spos * P + dcol] = 1.0

    def wrap(flat, ntiles):
        if ntiles == 0:
            return np.zeros((N_CORES, P, 0), np.int16)
        a = flat.reshape(N_CORES, ntiles * 8, 16).transpose(0, 2, 1)
        return np.ascontiguousarray(np.tile(a, (1, 8, 1)))

    meta = (
        tuple(int(v) for v in segtiles_e),
        tuple(int(v) for v in segtiles_o),
        tuple(int(v) for v in lo[:, 0]),
        tuple(int(v) for v in hi[:, 0]),
        tuple(int(v) for v in lo[:, 1]),
        tuple(int(v) for v in hi[:, 1]),
        tuple(int(v) for v in soff),
        tuple(calls),
        cnts_cols,
    )
    return meta, deg, wrap(idxe_flat, sTL), wrap(idxo_flat, sTH), scon, cnts


def _build_program(meta, generic_affine):
    import concourse.bass as bass
    import concourse.tile as tile
    from concourse import bacc as bacc_mod
    from concourse import mybir
    from contextlib import ExitStack

    f32 = mybir.dt.float32
    bf16 = mybir.dt.bfloat16
    fp8 = mybir.dt.float8e4
    i16 = mybir.dt.int16
    Alu = mybir.AluOpType
    Act = mybir.ActivationFunctionType

    segtiles_e, segtiles_o, lo_e, hi_e, lo_o, hi_o, soff, calls, cnts_cols = meta
    n_segs = len(segtiles_e)
    seg_of = [bb // SEG_BLOCKS for bb in range(N_BLOCKS)]
    egoff = np.concatenate([[0], np.cumsum(segtiles_e)]).astype(int)
    ogoff = np.concatenate([[0], np.cumsum(segtiles_o)]).astype(int)
    sTL, sTH = int(egoff[-1]), int(ogoff[-1])
    s_tiles_tot = int(soff[-1])

    W2 = WIDTH
    BW = 2 * (WIDTH + 1) + P
    VW = (WIDTH + 1) + N_BLOCKS * P
    NW = WIDTH + 1  # 257

    nc = bacc_mod.Bacc(None, target_bir_lowering=False, debug=False, num_swdge_queues=4)
    xe_d = nc.declare_dram_parameter("xe", [HALF, WIDTH], bf16, isOutput=False)
    xo_d = nc.declare_dram_parameter("xo", [HALF, WIDTH], bf16, isOutput=False)
    idxe_d = nc.declare_dram_parameter("idxe", [P, 8 * sTL], i16, isOutput=False)
    idxo_d = nc.declare_dram_parameter("idxo", [P, 8 * sTH], i16, isOutput=False)
    scon_d = nc.declare_dram_parameter("scon", [P, s_tiles_tot * P], fp8, isOutput=False)
    xself_d = nc.declare_dram_parameter("xself", [P, N_BLOCKS * W2], bf16, isOutput=False)
    fcon_d = nc.declare_dram_parameter("fcon", [P, N_BLOCKS], f32, isOutput=False)
    wcon_d = nc.declare_dram_parameter("wcon", [P, BW], bf16, isOutput=False)
    vrow_d = nc.declare_dram_parameter("vrow", [1, VW], bf16, isOutput=False)
    cnts_d = nc.declare_dram_parameter("cnts", [1, max(1, cnts_cols)], mybir.dt.int32, isOutput=False)
    if generic_affine:
        gb_d = nc.declare_dram_parameter("gbcon", [P, 2 * WIDTH], f32, isOutput=False)
    out_d = nc.declare_dram_parameter("out", [NODES_PER_CORE, WIDTH], f32, isOutput=True)

    with tile.TileContext(nc) as tc:
        with ExitStack() as ctx:
            const = ctx.enter_context(tc.tile_pool(name="const", bufs=1))
            GPOOL_BUFS = 6
            gpool = ctx.enter_context(tc.tile_pool(name="g", bufs=GPOOL_BUFS))
            spool = ctx.enter_context(tc.tile_pool(name="s", bufs=4))
            apool = ctx.enter_context(tc.tile_pool(name="aggT", bufs=3))
            ypool = ctx.enter_context(tc.tile_pool(name="y", bufs=2))
            sqpool = ctx.enter_context(tc.tile_pool(name="sq", bufs=2))
            stat = ctx.enter_context(tc.tile_pool(name="stat", bufs=6))
            ppool = ctx.enter_context(tc.tile_pool(name="psA", bufs=3, space="PSUM"))
            opsum = ctx.enter_context(tc.tile_pool(name="psO", bufs=2, space="PSUM"))

            idxe_sb = const.tile([P, 8 * sTL], i16)
            nc.sync.dma_start(idxe_sb[:], idxe_d[:, :])
            idxo_sb = const.tile([P, 8 * sTH], i16)
            nc.sync.dma_start(idxo_sb[:], idxo_d[:, :])
            fcon_sb = const.tile([P, N_BLOCKS], f32)
            nc.sync.dma_start(fcon_sb[:], fcon_d[:, :])
            wcon_sb = const.tile([P, BW], bf16)
            nc.sync.dma_start(wcon_sb[:], wcon_d[:, :])
            vrow_sb = const.tile([1, VW], bf16)
            nc.sync.dma_start(vrow_sb[:], vrow_d[:, :])
            cnt_sb = const.tile([1, max(1, cnts_cols)], mybir.dt.int32)
            nc.sync.dma_start(cnt_sb[:], cnts_d[:, :])
            xself_sb = const.tile([P, N_BLOCKS * W2], bf16)
            nc.sync.dma_start(xself_sb[:], xself_d[:, :])
            if generic_affine:
                gb_sb = const.tile([P, 2 * WIDTH], f32)
                nc.sync.dma_start(gb_sb[:], gb_d[:, :])

            wt_sb = wcon_sb[:, : 2 * NW]
            ident_sb = wcon_sb[:, 2 * NW : 2 * NW + P]

            gregs = [
                list(nc.alloc_registers(f"gcnt{i}", engines=[mybir.EngineType.Pool]))[0]
                for i in range(2)
            ]
            calls_by_seg = {}
            for cl in calls:
                calls_by_seg.setdefault(cl[0], []).append(cl)
            max_ne = max(segtiles_e)
            max_no = max(segtiles_o)
            qn = 0
            rn = 0
            for s in range(n_segs):
                ne, no = segtiles_e[s], segtiles_o[s]
                blocks = [bb for bb in range(N_BLOCKS) if seg_of[bb] == s]
                ge = go = None
                QORD = (0, 2, 1, 3)
                # uniform max-sized tiles; first use of each rotating buffer is
                # zeroed (split across DVE+ACT) so slots skipped by truncated
                # gathers never expose uninitialized SBUF (NaN*0 = NaN)
                if ne:
                    ge = gpool.tile([P, max_ne, WIDTH], bf16, tag="ge")
                    if s < GPOOL_BUFS:
                        h = max_ne // 2
                        nc.vector.memset(ge[:, 0:h, :], 0.0)
                        nc.scalar.memzero(ge[:, h:max_ne, :])
                if no:
                    go = gpool.tile([P, max_no, WIDTH], bf16, tag="go")
                    if s < GPOOL_BUFS:
                        h = max_no // 2
                        nc.vector.memset(go[:, 0:h, :], 0.0)
                        nc.scalar.memzero(go[:, h:max_no, :])
                for _, par, c0, c1, ridx in calls_by_seg.get(s, []):
                    gt = ge if par == 0 else go
                    tbl = xe_d if par == 0 else xo_d
                    isb = idxe_sb if par == 0 else idxo_sb
                    goff = int(egoff[s]) if par == 0 else int(ogoff[s])
                    if ridx >= 0:
                        reg = gregs[rn % 2]
                        rn += 1
                        nc.reg_load(reg, cnt_sb[0:1, ridx : ridx + 1])
                        nreg = reg
                    else:
                        nreg = (c1 - c0) * P
                    nc.gpsimd.dma_gather(
                        gt[:, c0:c1, :],
                        tbl[:, :],
                        isb[:, 8 * (goff + c0) : 8 * (goff + c1)],
                        (c1 - c0) * P,
                        nreg,
                        WIDTH,
                        queue_num=QORD[qn % 4],
                    )
                    qn += 1
                st0 = int(soff[blocks[0]])
                nst = int(soff[blocks[-1] + 1]) - st0
                sc = spool.tile([P, nst * P], fp8, tag="sc")
                # ACT-ring HWDGE: keeps the S-stream prefetch out of the Sync
                # queue, where it would sit behind output stores that wait on
                # compute (head-of-line blocking starves the PE mid-run)
                nc.scalar.dma_start(sc[:], scon_d[:, st0 * P : (st0 + nst) * P])
                for bb in blocks:
                    seq = [(ge, tt) for tt in range(lo_e[bb], hi_e[bb])] + [
                        (go, tt) for tt in range(lo_o[bb], hi_o[bb])
                    ]
                    ntb = len(seq)
                    assert ntb > 0
                    ps0 = ppool.tile([P, P], f32, tag="ps0")
                    ps1 = ppool.tile([P, P], f32, tag="ps1")
                    for k, (gt, col) in enumerate(seq):
                        s_ap = sc[:, (int(soff[bb]) - st0 + k) * P : (int(soff[bb]) - st0 + k + 1) * P]
                        nc.tensor.matmul(
                            out=ps0[:],
                            lhsT=gt[:, col, 0:P],
                            rhs=s_ap,
                            start=(k == 0),
                            stop=(k == ntb - 1),
                        )
                        nc.tensor.matmul(
                            out=ps1[:],
                            lhsT=gt[:, col, P:WIDTH],
                            rhs=s_ap,
                            start=(k == 0),
                            stop=(k == ntb - 1),
                        )
                    # PSUM -> SBUF cast; the self-loop term (dinv*x_self)^T
                    # rides along as a free tensor_tensor add
                    a = apool.tile([P, 2 * P], bf16, tag="a")
                    nc.vector.tensor_tensor(
                        out=a[:, 0:P],
                        in0=ps0[:],
                        in1=xself_sb[:, bb * W2 : bb * W2 + P],
                        op=Alu.add,
                    )
                    nc.vector.tensor_tensor(
                        out=a[:, P : 2 * P],
                        in0=ps1[:],
                        in1=xself_sb[:, bb * W2 + P : (bb + 1) * W2],
                        op=Alu.add,
                    )
                    po = opsum.tile([P, NW], f32, tag="po")
                    nc.tensor.matmul(
                        out=po[:],
                        lhsT=a[:, 0:P],
                        rhs=wt_sb[:, 0:NW],
                        start=True,
                        stop=False,
                    )
                    nc.tensor.matmul(
                        out=po[:],
                        lhsT=a[:, P : 2 * P],
                        rhs=wt_sb[:, NW : 2 * NW],
                        start=False,
                        stop=False,
                    )
                    nc.tensor.matmul(
                        out=po[:],
                        lhsT=vrow_sb[:, NW + bb * P : NW + (bb + 1) * P],
                        rhs=vrow_sb[:, 0:NW],
                        start=False,
                        stop=True,
                    )
                    sq = sqpool.tile([P, WIDTH], f32, tag="sq")
                    ssq = stat.tile([P, 1], f32, tag="ssq")
                    nc.scalar.activation(
                        out=sq[:], in_=po[:, :WIDTH], func=Act.Square, accum_out=ssq[:]
                    )
                    m2 = stat.tile([P, 1], f32, tag="m2")
                    nc.scalar.activation(
                        out=m2[:],
                        in_=po[:, WIDTH : WIDTH + 1],
                        func=Act.Square,
                        scale=1.0 / WIDTH,
                    )
                    rv = stat.tile([P, 1], f32, tag="rv")
                    nc.vector.tensor_scalar(
                        out=rv[:],
                        in0=ssq[:],
                        scalar1=1.0 / WIDTH,
                        scalar2=m2[:, :1],
                        op0=Alu.mult,
                        op1=Alu.subtract,
                    )
                    sd = stat.tile([P, 1], f32, tag="sd")
                    nc.scalar.activation(
                        out=sd[:], in_=rv[:], func=Act.Sqrt, bias=fcon_sb[:, bb : bb + 1]
                    )
                    rstd = stat.tile([P, 1], f32, tag="rstd")
                    nc.vector.reciprocal(rstd[:], sd[:])
                    nb = stat.tile([P, 1], f32, tag="nb")
                    nc.vector.tensor_scalar(
                        out=nb[:],
                        in0=po[:, WIDTH : WIDTH + 1],
                        scalar1=-1.0 / WIDTH,
                        scalar2=rstd[:, :1],
                        op0=Alu.mult,
                        op1=Alu.mult,
                    )
                    yo = ypool.tile([P, WIDTH], f32, tag="yo")
                    if not generic_affine:
                        nc.scalar.activation(
                            out=yo[:],
                            in_=po[:, :WIDTH],
                            func=Act.Relu,
                            scale=rstd[:, :1],
                            bias=nb[:, :1],
                        )
                    else:
                        t1 = ypool.tile([P, WIDTH], f32, tag="t1")
                        nc.scalar.activation(
                            out=t1[:],
                            in_=po[:, :WIDTH],
                            func=Act.Identity,
                            scale=rstd[:, :1],
                            bias=nb[:, :1],
                        )
                        t2 = ypool.tile([P, WIDTH], f32, tag="t2")
                        nc.vector.tensor_tensor(
                            out=t2[:], in0=t1[:], in1=gb_sb[:, :WIDTH], op=Alu.mult
                        )
                        t3 = ypool.tile([P, WIDTH], f32, tag="t3")
                        nc.vector.tensor_tensor(
                            out=t3[:], in0=t2[:], in1=gb_sb[:, WIDTH:], op=Alu.add
                        )
                        nc.scalar.activation(out=yo[:], in_=t3[:], func=Act.Relu)
                    rows = min(P, NODES_PER_CORE - bb * P)
                    nc.sync.dma_start(out_d[bb * P : bb * P + rows, :], yo[:rows, :])
    return nc


def _pack_inputs(meta, deg, idxe, idxo, scon, cnts, x, W, bias, gamma, beta, generic_affine):
    bfnp = ml_dtypes.bfloat16

    dinv = (1.0 / np.sqrt(deg)).astype(np.float64)
    xs = (np.asarray(x, np.float64) * dinv[:, None]).astype(bfnp)
    xe = np.ascontiguousarray(xs[0::2])
    xo = np.ascontiguousarray(xs[1::2])

    # transposed self tiles: xself[c][p, b*256 + h*128 + d] = xs_pad[b*128+d, h*128+p]
    xself_all = np.zeros((N_CORES, P, N_BLOCKS * WIDTH), bfnp)
    for c in range(N_CORES):
        sl = xs[c * NODES_PER_CORE : (c + 1) * NODES_PER_CORE]
        flat = np.zeros((N_BLOCKS * P, WIDTH), bfnp)
        flat[: NODES_PER_CORE] = sl
        xself_all[c] = np.ascontiguousarray(
            flat.reshape(N_BLOCKS, P, 2, P)
            .transpose(3, 0, 2, 1)
            .reshape(P, N_BLOCKS * WIDTH)
        )

    degp = np.ones((N_CORES, N_BLOCKS * P), np.float64)
    for c in range(N_CORES):
        degp[c, :NODES_PER_CORE] = deg[c * NODES_PER_CORE : (c + 1) * NODES_PER_CORE]
    epsdeg = (LN_EPS * degp).astype(np.float32).reshape(N_CORES, N_BLOCKS, P)
    epsdeg = np.ascontiguousarray(epsdeg.transpose(0, 2, 1))

    WT32 = np.asarray(W, np.float64).T
    rs = WT32.sum(axis=1, keepdims=True)
    WTe = np.concatenate([WT32, rs], axis=1).astype(bfnp)
    wt = np.concatenate([WTe[:P], WTe[P:]], axis=1)
    ident = np.eye(P, dtype=bfnp)
    wcon = np.ascontiguousarray(np.concatenate([wt, ident], axis=1))

    b64 = np.asarray(bias, np.float64)
    brow = np.concatenate([b64, [b64.sum()]])
    sdeg = np.sqrt(degp)
    vrow_all = np.concatenate(
        [np.tile(brow[None, :], (N_CORES, 1)), sdeg], axis=1
    ).astype(bfnp)

    in_maps = []
    for c in range(N_CORES):
        m = {
            "xe": xe,
            "xo": xo,
            "idxe": np.ascontiguousarray(idxe[c]),
            "idxo": np.ascontiguousarray(idxo[c]),
            "scon": np.ascontiguousarray(scon[c]),
            "xself": xself_all[c],
            "fcon": epsdeg[c],
            "wcon": wcon,
            "vrow": vrow_all[c : c + 1],
            "cnts": np.ascontiguousarray(cnts[c]),
        }
        if generic_affine:
            gb = np.concatenate(
                [
                    np.tile(np.asarray(gamma, np.float32)[None, :], (P, 1)),
                    np.tile(np.asarray(beta, np.float32)[None, :], (P, 1)),
                ],
                axis=1,
            )
            m["gbcon"] = np.ascontiguousarray(gb)
        in_maps.append(m)
    return in_maps


_PROGRAM_CACHE = {}


def kernel(x, edge_index, W, b, gamma, beta, _run_kwargs=None):
    from concourse.bass_utils import run_bass_kernel_spmd

    x = np.asarray(x)
    W = np.asarray(W)
    bias = np.asarray(b)
    gamma = np.asarray(gamma)
    beta = np.asarray(beta)

    meta, deg, idxe, idxo, scon, cnts = _preprocess(edge_index)
    generic_affine = not (np.all(gamma == 1.0) and np.all(beta == 0.0))

    key = (meta, generic_affine)
    if key not in _PROGRAM_CACHE:
        nc = _build_program(meta, generic_affine)
        nc.finalize()
        _PROGRAM_CACHE[key] = nc
    nc = _PROGRAM_CACHE[key]

    in_maps = _pack_inputs(
        meta, deg, idxe, idxo, scon, cnts, x, W, bias, gamma, beta, generic_affine
    )

    kwargs = dict(_run_kwargs or {})
    kwargs.pop("_result", None)
    rr = run_bass_kernel_spmd(nc, in_maps, list(range(N_CORES)), **kwargs)
    out = np.concatenate([rr.results[c]["out"] for c in range(N_CORES)], axis=0)
    if _run_kwargs is not None:
        _run_kwargs["_result"] = rr
    return np.ascontiguousarray(out.astype(np.float32))


# revision 24
# speedup vs baseline: 1.0793x; 1.0793x over previous
"""GCN block (GCNConv + LayerNorm + ReLU) on 8 Trainium2 NeuronCores.

v3: like v2 (precomputed fp8 one-hot S stream, dinv-prescaled gather
tables, identity-rhs self-loops, fused bias/LN/ReLU epilogue) but the
message streams are PACKED: within a segment of SEG_BLOCKS dst blocks,
each core's messages are laid out back-to-back with NO per-block tile
padding (padding only at segment ends, to the max core's tile count).
Block -> tile mapping becomes data-dependent, so each block's PSUM
accumulation covers the UNION across cores of the tiles its messages can
land in (a static interval, from the per-core cumulative counts); the
per-core S tiles zero out the slots that belong to other blocks.
"""

import math
import sys

sys.path.insert(0, "/opt/trn_rl_repo")

import numpy as np
import ml_dtypes

N_NODES = 50000
WIDTH = 256
N_CORES = 8
NODES_PER_CORE = N_NODES // N_CORES  # 6250
P = 128
N_BLOCKS = math.ceil(NODES_PER_CORE / P)  # 49 (last block has 106 rows)
LN_EPS = 1e-5
HALF = N_NODES // 2  # rows per gather table
SEG_BLOCKS = 4  # dst blocks packed per gather segment
GATHER_TILE_CAP = 8  # max tiles (128 idxs) per dma_gather call (HW limit 1024)


def _preprocess(edge_index):
    """Pack non-self-loop messages per (core, segment, parity) contiguously.

    Returns (meta, deg, idxe, idxo, scon) where meta carries the static
    structure shared by all cores:
      meta = (segtiles_e, segtiles_o,   # [n_segs] tiles per segment/parity
              lo_e, hi_e, lo_o, hi_o,   # [N_BLOCKS] participation intervals
                                        # (tile idx relative to segment start)
              soff)                     # [N_BLOCKS] S-stream tile offset
    """
    src = np.asarray(edge_index[0]).astype(np.int64)
    dst = np.asarray(edge_index[1]).astype(np.int64)

    deg = (np.bincount(dst, minlength=N_NODES) + 1).astype(np.float64)  # + self loop

    core = dst // NODES_PER_CORE
    r = dst % NODES_PER_CORE
    blk = np.minimum(r // P, N_BLOCKS - 1)
    dcol = r - blk * P
    tab = src & 1
    gbin = (core * N_BLOCKS + blk) * 2 + tab

    order = np.argsort(gbin, kind="stable")
    src, dcol, gbin = src[order], dcol[order], gbin[order]
    c = gbin // (N_BLOCKS * 2)
    b = (gbin // 2) % N_BLOCKS
    t = gbin & 1

    cnt = np.bincount(gbin, minlength=N_CORES * N_BLOCKS * 2).reshape(
        N_CORES, N_BLOCKS, 2
    )
    n_segs = math.ceil(N_BLOCKS / SEG_BLOCKS)
    seg_of = np.arange(N_BLOCKS) // SEG_BLOCKS

    # per-core cumulative counts within each segment -> slot of each message
    # start[c, b, t] = within-segment start slot of block b's bucket
    start = np.zeros((N_CORES, N_BLOCKS, 2), np.int64)
    for s in range(n_segs):
        bs = np.where(seg_of == s)[0]
        cum = np.cumsum(cnt[:, bs, :], axis=1)
        start[:, bs[1:], :] = cum[:, :-1, :]
    end = start + cnt  # within-segment end slot

    segtiles = np.zeros((n_segs, 2), np.int64)
    for s in range(n_segs):
        bs = np.where(seg_of == s)[0]
        tot = cnt[:, bs, :].sum(axis=1)  # [8, 2]
        segtiles[s] = np.ceil(tot.max(axis=0) / P).astype(np.int64)
    segtiles_e = segtiles[:, 0]
    segtiles_o = segtiles[:, 1]

    # participation intervals (tiles relative to segment start)
    lo = np.zeros((N_BLOCKS, 2), np.int64)
    hi = np.zeros((N_BLOCKS, 2), np.int64)
    for bb in range(N_BLOCKS):
        s = seg_of[bb]
        for tt in range(2):
            lo[bb, tt] = start[:, bb, tt].min() // P
            hi[bb, tt] = min(
                int(np.ceil(end[:, bb, tt].max() / P)), int(segtiles[s, tt])
            )
            hi[bb, tt] = max(hi[bb, tt], lo[bb, tt])  # empty-bucket guard
    we = (hi - lo)[:, 0]
    wo = (hi - lo)[:, 1]
    soff = np.concatenate([[0], np.cumsum(we + wo)])  # S tile offset per block
    s_tiles_tot = int(soff[-1])

    # gather-stream tile offsets per segment
    egoff = np.concatenate([[0], np.cumsum(segtiles_e)])
    ogoff = np.concatenate([[0], np.cumsum(segtiles_o)])
    sTL, sTH = int(egoff[-1]), int(ogoff[-1])

    # per-message placement
    starts_flat = start[c, b, t]  # within-seg start slot of this bucket
    jj = np.zeros(len(gbin), np.int64)
    bstarts = np.concatenate([[0], np.cumsum(cnt.ravel())])[:-1]
    jj = np.arange(len(gbin)) - bstarts[gbin]  # index within bucket
    slot = starts_flat + jj  # within-segment slot
    tile_in_seg = slot // P
    p = slot % P

    idxe_flat = np.zeros((N_CORES, sTL * P), np.int16)
    idxo_flat = np.zeros((N_CORES, sTH * P), np.int16)
    scon = np.zeros((N_CORES, P, s_tiles_tot * P), ml_dtypes.float8_e4m3)

    idx16 = (src >> 1).astype(np.int16)
    seg_m = seg_of[b]
    ev = t == 0
    Je = (egoff[seg_m] + tile_in_seg) * P + p
    Jo = (ogoff[seg_m] + tile_in_seg) * P + p
    idxe_flat[c[ev], Je[ev]] = idx16[ev]
    idxo_flat[c[~ev], Jo[~ev]] = idx16[~ev]

    # per-(core, seg, parity) real counts; tail pads become -1 so a
    # register-supplied num_idxs lets the Q7 skip generating them
    segreal = np.zeros((N_CORES, n_segs, 2), np.int64)
    for s in range(n_segs):
        bs = np.where(seg_of == s)[0]
        segreal[:, s, :] = cnt[:, bs, :].sum(axis=1)
    for cc in range(N_CORES):
        for s in range(n_segs):
            r0 = int(segreal[cc, s, 0])
            idxe_flat[cc, int(egoff[s]) * P + r0 : int(egoff[s + 1]) * P] = -1
            r1 = int(segreal[cc, s, 1])
            idxo_flat[cc, int(ogoff[s]) * P + r1 : int(ogoff[s + 1]) * P] = -1

    # static call plan: CAP-sized windows; a window entirely below every
    # core's real count gets a static count, the rest read a register
    calls = []
    cnts_cols = 0
    for s in range(n_segs):
        for par, tiles in ((0, int(segtiles_e[s])), (1, int(segtiles_o[s]))):
            full = int(segreal[:, s, par].min()) // P
            c0 = 0
            while c0 < tiles:
                c1 = min(c0 + GATHER_TILE_CAP, tiles)
                if c1 <= full:
                    calls.append((s, par, c0, c1, -1))
                else:
                    calls.append((s, par, c0, c1, cnts_cols))
                    cnts_cols += 1
                c0 = c1
    cnts = np.zeros((N_CORES, 1, max(1, cnts_cols)), np.int32)
    for s, par, c0, c1, ridx in calls:
        if ridx >= 0:
            v = np.clip(segreal[:, s, par] - c0 * P, 1, (c1 - c0) * P)
            cnts[:, 0, ridx] = v.astype(np.int32)

    # S stream: per block, even participation tiles then odd
    spos = np.where(
        ev,
        soff[b] + (tile_in_seg - lo[b, 0]),
        soff[b] + we[b] + (tile_in_seg - lo[b, 1]),
    )
    scon[c, p, spos * P + dcol] = 1.0

    def wrap(flat, ntiles):
        if ntiles == 0:
            return np.zeros((N_CORES, P, 0), np.int16)
        a = flat.reshape(N_CORES, ntiles * 8, 16).transpose(0, 2, 1)
        return np.ascontiguousarray(np.tile(a, (1, 8, 1)))

    meta = (
        tuple(int(v) for v in segtiles_e),
        tuple(int(v) for v in segtiles_o),
        tuple(int(v) for v in lo[:, 0]),
        tuple(int(v) for v in hi[:, 0]),
        tuple(int(v) for v in lo[:, 1]),
        tuple(int(v) for v in hi[:, 1]),
        tuple(int(v) for v in soff),
        tuple(calls),
        cnts_cols,
    )
    return meta, deg, wrap(idxe_flat, sTL), wrap(idxo_flat, sTH), scon, cnts


def _build_program(meta, generic_affine):
    import concourse.bass as bass
    import concourse.tile as tile
    from concourse import bacc as bacc_mod
    from concourse import mybir
    from contextlib import ExitStack

    f32 = mybir.dt.float32
    bf16 = mybir.dt.bfloat16
    fp8 = mybir.dt.float8e4
    i16 = mybir.dt.int16
    Alu = mybir.AluOpType
    Act = mybir.ActivationFunctionType

    segtiles_e, segtiles_o, lo_e, hi_e, lo_o, hi_o, soff, calls, cnts_cols = meta
    n_segs = len(segtiles_e)
    seg_of = [bb // SEG_BLOCKS for bb in range(N_BLOCKS)]
    egoff = np.concatenate([[0], np.cumsum(segtiles_e)]).astype(int)
    ogoff = np.concatenate([[0], np.cumsum(segtiles_o)]).astype(int)
    sTL, sTH = int(egoff[-1]), int(ogoff[-1])
    s_tiles_tot = int(soff[-1])

    W2 = WIDTH
    BW = 2 * (WIDTH + 1) + P
    VW = (WIDTH + 1) + N_BLOCKS * P
    NW = WIDTH + 1  # 257

    nc = bacc_mod.Bacc(None, target_bir_lowering=False, debug=False, num_swdge_queues=4)
    xe_d = nc.declare_dram_parameter("xe", [HALF, WIDTH], bf16, isOutput=False)
    xo_d = nc.declare_dram_parameter("xo", [HALF, WIDTH], bf16, isOutput=False)
    idxe_d = nc.declare_dram_parameter("idxe", [P, 8 * sTL], i16, isOutput=False)
    idxo_d = nc.declare_dram_parameter("idxo", [P, 8 * sTH], i16, isOutput=False)
    scon_d = nc.declare_dram_parameter("scon", [P, s_tiles_tot * P], fp8, isOutput=False)
    xself_d = nc.declare_dram_parameter("xself", [P, N_BLOCKS * W2], bf16, isOutput=False)
    fcon_d = nc.declare_dram_parameter("fcon", [P, N_BLOCKS], f32, isOutput=False)
    wcon_d = nc.declare_dram_parameter("wcon", [P, BW], bf16, isOutput=False)
    vrow_d = nc.declare_dram_parameter("vrow", [1, VW], bf16, isOutput=False)
    cnts_d = nc.declare_dram_parameter("cnts", [1, max(1, cnts_cols)], mybir.dt.int32, isOutput=False)
    if generic_affine:
        gb_d = nc.declare_dram_parameter("gbcon", [P, 2 * WIDTH], f32, isOutput=False)
    out_d = nc.declare_dram_parameter("out", [NODES_PER_CORE, WIDTH], f32, isOutput=True)

    with tile.TileContext(nc) as tc:
        with ExitStack() as ctx:
            const = ctx.enter_context(tc.tile_pool(name="const", bufs=1))
            GPOOL_BUFS = 6
            gpool = ctx.enter_context(tc.tile_pool(name="g", bufs=GPOOL_BUFS))
            spool = ctx.enter_context(tc.tile_pool(name="s", bufs=4))
            apool = ctx.enter_context(tc.tile_pool(name="aggT", bufs=3))
            ypool = ctx.enter_context(tc.tile_pool(name="y", bufs=2))
            sqpool = ctx.enter_context(tc.tile_pool(name="sq", bufs=2))
            stat = ctx.enter_context(tc.tile_pool(name="stat", bufs=6))
            ppool = ctx.enter_context(tc.tile_pool(name="psA", bufs=3, space="PSUM"))
            opsum = ctx.enter_context(tc.tile_pool(name="psO", bufs=2, space="PSUM"))

            idxe_sb = const.tile([P, 8 * sTL], i16)
            nc.sync.dma_start(idxe_sb[:], idxe_d[:, :])
            idxo_sb = const.tile([P, 8 * sTH], i16)
            nc.sync.dma_start(idxo_sb[:], idxo_d[:, :])
            fcon_sb = const.tile([P, N_BLOCKS], f32)
            nc.sync.dma_start(fcon_sb[:], fcon_d[:, :])
            wcon_sb = const.tile([P, BW], bf16)
            nc.sync.dma_start(wcon_sb[:], wcon_d[:, :])
            vrow_sb = const.tile([1, VW], bf16)
            nc.sync.dma_start(vrow_sb[:], vrow_d[:, :])
            cnt_sb = const.tile([1, max(1, cnts_cols)], mybir.dt.int32)
            nc.sync.dma_start(cnt_sb[:], cnts_d[:, :])
            xself_sb = const.tile([P, N_BLOCKS * W2], bf16)
            nc.sync.dma_start(xself_sb[:], xself_d[:, :])
            if generic_affine:
                gb_sb = const.tile([P, 2 * WIDTH], f32)
                nc.sync.dma_start(gb_sb[:], gb_d[:, :])

            wt_sb = wcon_sb[:, : 2 * NW]
            ident_sb = wcon_sb[:, 2 * NW : 2 * NW + P]

            seg_blocks_of = {}
            for bb in range(N_BLOCKS):
                seg_blocks_of.setdefault(seg_of[bb], []).append(bb)
            sc_tiles = {}

            def _load_sc(sp):
                bl = seg_blocks_of[sp]
                st0 = int(soff[bl[0]])
                nst = int(soff[bl[-1] + 1]) - st0
                t = spool.tile([P, nst * P], fp8, tag="sc")
                nc.sync.dma_start(t[:], scon_d[:, st0 * P : (st0 + nst) * P])
                sc_tiles[sp] = (t, st0)

            gregs = [
                list(nc.alloc_registers(f"gcnt{i}", engines=[mybir.EngineType.Pool]))[0]
                for i in range(2)
            ]
            calls_by_seg = {}
            for cl in calls:
                calls_by_seg.setdefault(cl[0], []).append(cl)
            max_ne = max(segtiles_e)
            max_no = max(segtiles_o)
            qn = 0
            rn = 0
            for s in range(n_segs):
                ne, no = segtiles_e[s], segtiles_o[s]
                blocks = [bb for bb in range(N_BLOCKS) if seg_of[bb] == s]
                ge = go = None
                QORD = (0, 2, 1, 3)
                # uniform max-sized tiles; first use of each rotating buffer is
                # zeroed (split across DVE+ACT) so slots skipped by truncated
                # gathers never expose uninitialized SBUF (NaN*0 = NaN)
                if ne:
                    ge = gpool.tile([P, max_ne, WIDTH], bf16, tag="ge")
                    if s < GPOOL_BUFS:
                        h = max_ne // 2
                        nc.vector.memset(ge[:, 0:h, :], 0.0)
                        nc.scalar.memzero(ge[:, h:max_ne, :])
                if no:
                    go = gpool.tile([P, max_no, WIDTH], bf16, tag="go")
                    if s < GPOOL_BUFS:
                        h = max_no // 2
                        nc.vector.memset(go[:, 0:h, :], 0.0)
                        nc.scalar.memzero(go[:, h:max_no, :])
                for _, par, c0, c1, ridx in calls_by_seg.get(s, []):
                    gt = ge if par == 0 else go
                    tbl = xe_d if par == 0 else xo_d
                    isb = idxe_sb if par == 0 else idxo_sb
                    goff = int(egoff[s]) if par == 0 else int(ogoff[s])
                    if ridx >= 0:
                        reg = gregs[rn % 2]
                        rn += 1
                        nc.reg_load(reg, cnt_sb[0:1, ridx : ridx + 1])
                        nreg = reg
                    else:
                        nreg = (c1 - c0) * P
                    nc.gpsimd.dma_gather(
                        gt[:, c0:c1, :],
                        tbl[:, :],
                        isb[:, 8 * (goff + c0) : 8 * (goff + c1)],
                        (c1 - c0) * P,
                        nreg,
                        WIDTH,
                        queue_num=QORD[qn % 4],
                    )
                    qn += 1
                # S loads are hoisted two segments ahead so they enter the
                # Sync queue before the output stores that wait on compute
                # (head-of-line blocking would starve the PE mid-run)
                if s == 0:
                    for sp in range(min(3, n_segs)):
                        _load_sc(sp)
                elif s + 2 < n_segs:
                    _load_sc(s + 2)
                sc, st0 = sc_tiles.pop(s)
                for bb in blocks:
                    seq = [(ge, tt) for tt in range(lo_e[bb], hi_e[bb])] + [
                        (go, tt) for tt in range(lo_o[bb], hi_o[bb])
                    ]
                    ntb = len(seq)
                    assert ntb > 0
                    ps0 = ppool.tile([P, P], f32, tag="ps0")
                    ps1 = ppool.tile([P, P], f32, tag="ps1")
                    for k, (gt, col) in enumerate(seq):
                        s_ap = sc[:, (int(soff[bb]) - st0 + k) * P : (int(soff[bb]) - st0 + k + 1) * P]
                        nc.tensor.matmul(
                            out=ps0[:],
                            lhsT=gt[:, col, 0:P],
                            rhs=s_ap,
                            start=(k == 0),
                            stop=(k == ntb - 1),
                        )
                        nc.tensor.matmul(
                            out=ps1[:],
                            lhsT=gt[:, col, P:WIDTH],
                            rhs=s_ap,
                            start=(k == 0),
                            stop=(k == ntb - 1),
                        )
                    # PSUM -> SBUF cast; the self-loop term (dinv*x_self)^T
                    # rides along as a free tensor_tensor add
                    a = apool.tile([P, 2 * P], bf16, tag="a")
                    nc.vector.tensor_tensor(
                        out=a[:, 0:P],
                        in0=ps0[:],
                        in1=xself_sb[:, bb * W2 : bb * W2 + P],
                        op=Alu.add,
                    )
                    nc.vector.tensor_tensor(
                        out=a[:, P : 2 * P],
                        in0=ps1[:],
                        in1=xself_sb[:, bb * W2 + P : (bb + 1) * W2],
                        op=Alu.add,
                    )
                    po = opsum.tile([P, NW], f32, tag="po")
                    nc.tensor.matmul(
                        out=po[:],
                        lhsT=a[:, 0:P],
                        rhs=wt_sb[:, 0:NW],
                        start=True,
                        stop=False,
                    )
                    nc.tensor.matmul(
                        out=po[:],
                        lhsT=a[:, P : 2 * P],
                        rhs=wt_sb[:, NW : 2 * NW],
                        start=False,
                        stop=False,
                    )
                    nc.tensor.matmul(
                        out=po[:],
                        lhsT=vrow_sb[:, NW + bb * P : NW + (bb + 1) * P],
                        rhs=vrow_sb[:, 0:NW],
                        start=False,
                        stop=True,
                    )
                    sq = sqpool.tile([P, WIDTH], f32, tag="sq")
                    ssq = stat.tile([P, 1], f32, tag="ssq")
                    nc.scalar.activation(
                        out=sq[:], in_=po[:, :WIDTH], func=Act.Square, accum_out=ssq[:]
                    )
                    m2 = stat.tile([P, 1], f32, tag="m2")
                    nc.scalar.activation(
                        out=m2[:],
                        in_=po[:, WIDTH : WIDTH + 1],
                        func=Act.Square,
                        scale=1.0 / WIDTH,
                    )
                    rv = stat.tile([P, 1], f32, tag="rv")
                    nc.vector.tensor_scalar(
                        out=rv[:],
                        in0=ssq[:],
                        scalar1=1.0 / WIDTH,
                        scalar2=m2[:, :1],
                        op0=Alu.mult,
                        op1=Alu.subtract,
                    )
                    sd = stat.tile([P, 1], f32, tag="sd")
                    nc.scalar.activation(
                        out=sd[:], in_=rv[:], func=Act.Sqrt, bias=fcon_sb[:, bb : bb + 1]
                    )
                    rstd = stat.tile([P, 1], f32, tag="rstd")
                    nc.vector.reciprocal(rstd[:], sd[:])
                    nb = stat.tile([P, 1], f32, tag="nb")
                    nc.vector.tensor_scalar(
                        out=nb[:],
                        in0=po[:, WIDTH : WIDTH + 1],
                        scalar1=-1.0 / WIDTH,
                        scalar2=rstd[:, :1],
                        op0=Alu.mult,
                        op1=Alu.mult,
                    )
                    yo = ypool.tile([P, WIDTH], f32, tag="yo")
                    if not generic_affine:
                        nc.scalar.activation(
                            out=yo[:],
                            in_=po[:, :WIDTH],
                            func=Act.Relu,
                            scale=rstd[:, :1],
                            bias=nb[:, :1],
                        )
                    else:
                        t1 = ypool.tile([P, WIDTH], f32, tag="t1")
                        nc.scalar.activation(
                            out=t1[:],
                            in_=po[:, :WIDTH],
                            func=Act.Identity,
                            scale=rstd[:, :1],
                            bias=nb[:, :1],
                        )
                        t2 = ypool.tile([P, WIDTH], f32, tag="t2")
                        nc.vector.tensor_tensor(
                            out=t2[:], in0=t1[:], in1=gb_sb[:, :WIDTH], op=Alu.mult
                        )
                        t3 = ypool.tile([P, WIDTH], f32, tag="t3")
                        nc.vector.tensor_tensor(
                            out=t3[:], in0=t2[:], in1=gb_sb[:, WIDTH:], op=Alu.add
                        )
                        nc.scalar.activation(out=yo[:], in_=t3[:], func=Act.Relu)
                    rows = min(P, NODES_PER_CORE - bb * P)
                    nc.sync.dma_start(out_d[bb * P : bb * P + rows, :], yo[:rows, :])
    return nc


def _pack_inputs(meta, deg, idxe, idxo, scon, cnts, x, W, bias, gamma, beta, generic_affine):
    bfnp = ml_dtypes.bfloat16

    dinv = (1.0 / np.sqrt(deg)).astype(np.float64)
    xs = (np.asarray(x, np.float64) * dinv[:, None]).astype(bfnp)
    xe = np.ascontiguousarray(xs[0::2])
    xo = np.ascontiguousarray(xs[1::2])

    # transposed self tiles: xself[c][p, b*256 + h*128 + d] = xs_pad[b*128+d, h*128+p]
    xself_all = np.zeros((N_CORES, P, N_BLOCKS * WIDTH), bfnp)
    for c in range(N_CORES):
        sl = xs[c * NODES_PER_CORE : (c + 1) * NODES_PER_CORE]
        flat = np.zeros((N_BLOCKS * P, WIDTH), bfnp)
        flat[: NODES_PER_CORE] = sl
        xself_all[c] = np.ascontiguousarray(
            flat.reshape(N_BLOCKS, P, 2, P)
            .transpose(3, 0, 2, 1)
            .reshape(P, N_BLOCKS * WIDTH)
        )

    degp = np.ones((N_CORES, N_BLOCKS * P), np.float64)
    for c in range(N_CORES):
        degp[c, :NODES_PER_CORE] = deg[c * NODES_PER_CORE : (c + 1) * NODES_PER_CORE]
    epsdeg = (LN_EPS * degp).astype(np.float32).reshape(N_CORES, N_BLOCKS, P)
    epsdeg = np.ascontiguousarray(epsdeg.transpose(0, 2, 1))

    WT32 = np.asarray(W, np.float64).T
    rs = WT32.sum(axis=1, keepdims=True)
    WTe = np.concatenate([WT32, rs], axis=1).astype(bfnp)
    wt = np.concatenate([WTe[:P], WTe[P:]], axis=1)
    ident = np.eye(P, dtype=bfnp)
    wcon = np.ascontiguousarray(np.concatenate([wt, ident], axis=1))

    b64 = np.asarray(bias, np.float64)
    brow = np.concatenate([b64, [b64.sum()]])
    sdeg = np.sqrt(degp)
    vrow_all = np.concatenate(
        [np.tile(brow[None, :], (N_CORES, 1)), sdeg], axis=1
    ).astype(bfnp)

    in_maps = []
    for c in range(N_CORES):
        m = {
            "xe": xe,
            "xo": xo,
            "idxe": np.ascontiguousarray(idxe[c]),
            "idxo": np.ascontiguousarray(idxo[c]),
            "scon": np.ascontiguousarray(scon[c]),
            "xself": xself_all[c],
            "fcon": epsdeg[c],
            "wcon": wcon,
            "vrow": vrow_all[c : c + 1],
            "cnts": np.ascontiguousarray(cnts[c]),
        }
        if generic_affine:
            gb = np.concatenate(
                [
                    np.tile(np.asarray(gamma, np.float32)[None, :], (P, 1)),
                    np.tile(np.asarray(beta, np.float32)[None, :], (P, 1)),
                ],
                axis=1,
            )
            m["gbcon"] = np.ascontiguousarray(gb)
        in_maps.append(m)
    return in_maps


_PROGRAM_CACHE = {}


def kernel(x, edge_index, W, b, gamma, beta, _run_kwargs=None):
    from concourse.bass_utils import run_bass_kernel_spmd

    x = np.asarray(x)
    W = np.asarray(W)
    bias = np.asarray(b)
    gamma = np.asarray(gamma)
    beta = np.asarray(beta)

    meta, deg, idxe, idxo, scon, cnts = _preprocess(edge_index)
    generic_affine = not (np.all(gamma == 1.0) and np.all(beta == 0.0))

    key = (meta, generic_affine)
    if key not in _PROGRAM_CACHE:
        nc = _build_program(meta, generic_affine)
        nc.finalize()
        _PROGRAM_CACHE[key] = nc
    nc = _PROGRAM_CACHE[key]

    in_maps = _pack_inputs(
        meta, deg, idxe, idxo, scon, cnts, x, W, bias, gamma, beta, generic_affine
    )

    kwargs = dict(_run_kwargs or {})
    kwargs.pop("_result", None)
    rr = run_bass_kernel_spmd(nc, in_maps, list(range(N_CORES)), **kwargs)
    out = np.concatenate([rr.results[c]["out"] for c in range(N_CORES)], axis=0)
    if _run_kwargs is not None:
        _run_kwargs["_result"] = rr
    return np.ascontiguousarray(out.astype(np.float32))


# revision 25
# speedup vs baseline: 1.1865x; 1.0994x over previous
"""GCN block (GCNConv + LayerNorm + ReLU) on 8 Trainium2 NeuronCores.

v3: like v2 (precomputed fp8 one-hot S stream, dinv-prescaled gather
tables, identity-rhs self-loops, fused bias/LN/ReLU epilogue) but the
message streams are PACKED: within a segment of SEG_BLOCKS dst blocks,
each core's messages are laid out back-to-back with NO per-block tile
padding (padding only at segment ends, to the max core's tile count).
Block -> tile mapping becomes data-dependent, so each block's PSUM
accumulation covers the UNION across cores of the tiles its messages can
land in (a static interval, from the per-core cumulative counts); the
per-core S tiles zero out the slots that belong to other blocks.
"""

import math
import sys

sys.path.insert(0, "/opt/trn_rl_repo")

import numpy as np
import ml_dtypes

N_NODES = 50000
WIDTH = 256
N_CORES = 8
NODES_PER_CORE = N_NODES // N_CORES  # 6250
P = 128
N_BLOCKS = math.ceil(NODES_PER_CORE / P)  # 49 (last block has 106 rows)
LN_EPS = 1e-5
HALF = N_NODES // 2  # rows per gather table
SEG_BLOCKS = 4  # dst blocks packed per gather segment
GATHER_TILE_CAP = 8  # max tiles (128 idxs) per dma_gather call (HW limit 1024)


def _preprocess(edge_index):
    """Pack non-self-loop messages per (core, segment, parity) contiguously.

    Returns (meta, deg, idxe, idxo, scon) where meta carries the static
    structure shared by all cores:
      meta = (segtiles_e, segtiles_o,   # [n_segs] tiles per segment/parity
              lo_e, hi_e, lo_o, hi_o,   # [N_BLOCKS] participation intervals
                                        # (tile idx relative to segment start)
              soff)                     # [N_BLOCKS] S-stream tile offset
    """
    src = np.asarray(edge_index[0]).astype(np.int64)
    dst = np.asarray(edge_index[1]).astype(np.int64)

    deg = (np.bincount(dst, minlength=N_NODES) + 1).astype(np.float64)  # + self loop

    core = dst // NODES_PER_CORE
    r = dst % NODES_PER_CORE
    blk = np.minimum(r // P, N_BLOCKS - 1)
    dcol = r - blk * P
    tab = src & 1
    gbin = (core * N_BLOCKS + blk) * 2 + tab

    order = np.argsort(gbin, kind="stable")
    src, dcol, gbin = src[order], dcol[order], gbin[order]
    c = gbin // (N_BLOCKS * 2)
    b = (gbin // 2) % N_BLOCKS
    t = gbin & 1

    cnt = np.bincount(gbin, minlength=N_CORES * N_BLOCKS * 2).reshape(
        N_CORES, N_BLOCKS, 2
    )
    n_segs = math.ceil(N_BLOCKS / SEG_BLOCKS)
    seg_of = np.arange(N_BLOCKS) // SEG_BLOCKS

    # per-core cumulative counts within each segment -> slot of each message
    # start[c, b, t] = within-segment start slot of block b's bucket
    start = np.zeros((N_CORES, N_BLOCKS, 2), np.int64)
    for s in range(n_segs):
        bs = np.where(seg_of == s)[0]
        cum = np.cumsum(cnt[:, bs, :], axis=1)
        start[:, bs[1:], :] = cum[:, :-1, :]
    end = start + cnt  # within-segment end slot

    segtiles = np.zeros((n_segs, 2), np.int64)
    for s in range(n_segs):
        bs = np.where(seg_of == s)[0]
        tot = cnt[:, bs, :].sum(axis=1)  # [8, 2]
        segtiles[s] = np.ceil(tot.max(axis=0) / P).astype(np.int64)
    segtiles_e = segtiles[:, 0]
    segtiles_o = segtiles[:, 1]

    # participation intervals (tiles relative to segment start)
    lo = np.zeros((N_BLOCKS, 2), np.int64)
    hi = np.zeros((N_BLOCKS, 2), np.int64)
    for bb in range(N_BLOCKS):
        s = seg_of[bb]
        for tt in range(2):
            lo[bb, tt] = start[:, bb, tt].min() // P
            hi[bb, tt] = min(
                int(np.ceil(end[:, bb, tt].max() / P)), int(segtiles[s, tt])
            )
            hi[bb, tt] = max(hi[bb, tt], lo[bb, tt])  # empty-bucket guard
    we = (hi - lo)[:, 0]
    wo = (hi - lo)[:, 1]
    soff = np.concatenate([[0], np.cumsum(we + wo)])  # S tile offset per block
    s_tiles_tot = int(soff[-1])

    # gather-stream tile offsets per segment
    egoff = np.concatenate([[0], np.cumsum(segtiles_e)])
    ogoff = np.concatenate([[0], np.cumsum(segtiles_o)])
    sTL, sTH = int(egoff[-1]), int(ogoff[-1])

    # per-message placement
    starts_flat = start[c, b, t]  # within-seg start slot of this bucket
    jj = np.zeros(len(gbin), np.int64)
    bstarts = np.concatenate([[0], np.cumsum(cnt.ravel())])[:-1]
    jj = np.arange(len(gbin)) - bstarts[gbin]  # index within bucket
    slot = starts_flat + jj  # within-segment slot
    tile_in_seg = slot // P
    p = slot % P

    idxe_flat = np.zeros((N_CORES, sTL * P), np.int16)
    idxo_flat = np.zeros((N_CORES, sTH * P), np.int16)
    scon = np.zeros((N_CORES, P, s_tiles_tot * P), ml_dtypes.float8_e4m3)

    idx16 = (src >> 1).astype(np.int16)
    seg_m = seg_of[b]
    ev = t == 0
    Je = (egoff[seg_m] + tile_in_seg) * P + p
    Jo = (ogoff[seg_m] + tile_in_seg) * P + p
    idxe_flat[c[ev], Je[ev]] = idx16[ev]
    idxo_flat[c[~ev], Jo[~ev]] = idx16[~ev]

    # per-(core, seg, parity) real counts; tail pads become -1 so a
    # register-supplied num_idxs lets the Q7 skip generating them
    segreal = np.zeros((N_CORES, n_segs, 2), np.int64)
    for s in range(n_segs):
        bs = np.where(seg_of == s)[0]
        segreal[:, s, :] = cnt[:, bs, :].sum(axis=1)
    for cc in range(N_CORES):
        for s in range(n_segs):
            r0 = int(segreal[cc, s, 0])
            idxe_flat[cc, int(egoff[s]) * P + r0 : int(egoff[s + 1]) * P] = -1
            r1 = int(segreal[cc, s, 1])
            idxo_flat[cc, int(ogoff[s]) * P + r1 : int(ogoff[s + 1]) * P] = -1

    # static call plan: CAP-sized windows; a window entirely below every
    # core's real count gets a static count, the rest read a register
    calls = []
    cnts_cols = 0
    for s in range(n_segs):
        for par, tiles in ((0, int(segtiles_e[s])), (1, int(segtiles_o[s]))):
            full = int(segreal[:, s, par].min()) // P
            c0 = 0
            while c0 < tiles:
                c1 = min(c0 + GATHER_TILE_CAP, tiles)
                if c1 <= full:
                    calls.append((s, par, c0, c1, -1))
                else:
                    calls.append((s, par, c0, c1, cnts_cols))
                    cnts_cols += 1
                c0 = c1
    cnts = np.zeros((N_CORES, 1, max(1, cnts_cols)), np.int32)
    for s, par, c0, c1, ridx in calls:
        if ridx >= 0:
            v = np.clip(segreal[:, s, par] - c0 * P, 1, (c1 - c0) * P)
            cnts[:, 0, ridx] = v.astype(np.int32)

    # S stream: per block, even participation tiles then odd
    spos = np.where(
        ev,
        soff[b] + (tile_in_seg - lo[b, 0]),
        soff[b] + we[b] + (tile_in_seg - lo[b, 1]),
    )
    scon[c, p, spos * P + dcol] = 1.0

    def wrap(flat, ntiles):
        if ntiles == 0:
            return np.zeros((N_CORES, P, 0), np.int16)
        a = flat.reshape(N_CORES, ntiles * 8, 16).transpose(0, 2, 1)
        return np.ascontiguousarray(np.tile(a, (1, 8, 1)))

    meta = (
        tuple(int(v) for v in segtiles_e),
        tuple(int(v) for v in segtiles_o),
        tuple(int(v) for v in lo[:, 0]),
        tuple(int(v) for v in hi[:, 0]),
        tuple(int(v) for v in lo[:, 1]),
        tuple(int(v) for v in hi[:, 1]),
        tuple(int(v) for v in soff),
        tuple(calls),
        cnts_cols,
    )
    return meta, deg, wrap(idxe_flat, sTL), wrap(idxo_flat, sTH), scon, cnts


def _build_program(meta, generic_affine):
    import concourse.bass as bass
    import concourse.tile as tile
    from concourse import bacc as bacc_mod
    from concourse import mybir
    from contextlib import ExitStack

    f32 = mybir.dt.float32
    bf16 = mybir.dt.bfloat16
    fp8 = mybir.dt.float8e4
    i16 = mybir.dt.int16
    Alu = mybir.AluOpType
    Act = mybir.ActivationFunctionType

    segtiles_e, segtiles_o, lo_e, hi_e, lo_o, hi_o, soff, calls, cnts_cols = meta
    n_segs = len(segtiles_e)
    seg_of = [bb // SEG_BLOCKS for bb in range(N_BLOCKS)]
    egoff = np.concatenate([[0], np.cumsum(segtiles_e)]).astype(int)
    ogoff = np.concatenate([[0], np.cumsum(segtiles_o)]).astype(int)
    sTL, sTH = int(egoff[-1]), int(ogoff[-1])
    s_tiles_tot = int(soff[-1])

    W2 = WIDTH
    BW = 2 * (WIDTH + 1) + P
    VW = (WIDTH + 1) + N_BLOCKS * P
    NW = WIDTH + 1  # 257

    nc = bacc_mod.Bacc(None, target_bir_lowering=False, debug=False, num_swdge_queues=4)
    xe_d = nc.declare_dram_parameter("xe", [HALF, WIDTH], bf16, isOutput=False)
    xo_d = nc.declare_dram_parameter("xo", [HALF, WIDTH], bf16, isOutput=False)
    idxe_d = nc.declare_dram_parameter("idxe", [P, 8 * sTL], i16, isOutput=False)
    idxo_d = nc.declare_dram_parameter("idxo", [P, 8 * sTH], i16, isOutput=False)
    scon_d = nc.declare_dram_parameter("scon", [P, s_tiles_tot * P], fp8, isOutput=False)
    xself_d = nc.declare_dram_parameter("xself", [P, N_BLOCKS * W2], bf16, isOutput=False)
    fcon_d = nc.declare_dram_parameter("fcon", [P, N_BLOCKS], f32, isOutput=False)
    wcon_d = nc.declare_dram_parameter("wcon", [P, BW], bf16, isOutput=False)
    vrow_d = nc.declare_dram_parameter("vrow", [1, VW], bf16, isOutput=False)
    cnts_d = nc.declare_dram_parameter("cnts", [1, max(1, cnts_cols)], mybir.dt.int32, isOutput=False)
    if generic_affine:
        gb_d = nc.declare_dram_parameter("gbcon", [P, 2 * WIDTH], f32, isOutput=False)
    out_d = nc.declare_dram_parameter("out", [NODES_PER_CORE, WIDTH], f32, isOutput=True)

    with tile.TileContext(nc) as tc:
        with ExitStack() as ctx:
            const = ctx.enter_context(tc.tile_pool(name="const", bufs=1))
            GPOOL_BUFS = 6
            gpool = ctx.enter_context(tc.tile_pool(name="g", bufs=GPOOL_BUFS))
            spool = ctx.enter_context(tc.tile_pool(name="s", bufs=4))
            apool = ctx.enter_context(tc.tile_pool(name="aggT", bufs=4))
            ypool = ctx.enter_context(tc.tile_pool(name="y", bufs=3))
            sqpool = ctx.enter_context(tc.tile_pool(name="sq", bufs=3))
            stat = ctx.enter_context(tc.tile_pool(name="stat", bufs=6))
            ppool = ctx.enter_context(tc.tile_pool(name="psA", bufs=2, space="PSUM"))
            opsum = ctx.enter_context(tc.tile_pool(name="psO", bufs=3, space="PSUM"))

            idxe_sb = const.tile([P, 8 * sTL], i16)
            nc.sync.dma_start(idxe_sb[:], idxe_d[:, :])
            idxo_sb = const.tile([P, 8 * sTH], i16)
            nc.sync.dma_start(idxo_sb[:], idxo_d[:, :])
            fcon_sb = const.tile([P, N_BLOCKS], f32)
            nc.sync.dma_start(fcon_sb[:], fcon_d[:, :])
            wcon_sb = const.tile([P, BW], bf16)
            nc.sync.dma_start(wcon_sb[:], wcon_d[:, :])
            vrow_sb = const.tile([1, VW], bf16)
            nc.sync.dma_start(vrow_sb[:], vrow_d[:, :])
            cnt_sb = const.tile([1, max(1, cnts_cols)], mybir.dt.int32)
            nc.sync.dma_start(cnt_sb[:], cnts_d[:, :])
            xself_sb = const.tile([P, N_BLOCKS * W2], bf16)
            nc.sync.dma_start(xself_sb[:], xself_d[:, :])
            if generic_affine:
                gb_sb = const.tile([P, 2 * WIDTH], f32)
                nc.sync.dma_start(gb_sb[:], gb_d[:, :])

            wt_sb = wcon_sb[:, : 2 * NW]
            ident_sb = wcon_sb[:, 2 * NW : 2 * NW + P]

            seg_blocks_of = {}
            for bb in range(N_BLOCKS):
                seg_blocks_of.setdefault(seg_of[bb], []).append(bb)
            sc_tiles = {}

            def _load_sc(sp):
                bl = seg_blocks_of[sp]
                st0 = int(soff[bl[0]])
                nst = int(soff[bl[-1] + 1]) - st0
                t = spool.tile([P, nst * P], fp8, tag="sc")
                nc.sync.dma_start(t[:], scon_d[:, st0 * P : (st0 + nst) * P])
                sc_tiles[sp] = (t, st0)

            gregs = [
                list(nc.alloc_registers(f"gcnt{i}", engines=[mybir.EngineType.Pool]))[0]
                for i in range(2)
            ]
            calls_by_seg = {}
            for cl in calls:
                calls_by_seg.setdefault(cl[0], []).append(cl)
            max_ne = max(segtiles_e)
            max_no = max(segtiles_o)
            qn = 0
            rn = 0
            for s in range(n_segs):
                ne, no = segtiles_e[s], segtiles_o[s]
                blocks = [bb for bb in range(N_BLOCKS) if seg_of[bb] == s]
                ge = go = None
                QORD = (0, 2, 1, 3)
                # uniform max-sized tiles; first use of each rotating buffer is
                # zeroed (split across DVE+ACT) so slots skipped by truncated
                # gathers never expose uninitialized SBUF (NaN*0 = NaN)
                if ne:
                    ge = gpool.tile([P, max_ne, WIDTH], bf16, tag="ge")
                    if s < GPOOL_BUFS:
                        h = max_ne // 2
                        nc.vector.memset(ge[:, 0:h, :], 0.0)
                        nc.scalar.memzero(ge[:, h:max_ne, :])
                if no:
                    go = gpool.tile([P, max_no, WIDTH], bf16, tag="go")
                    if s < GPOOL_BUFS:
                        h = max_no // 2
                        nc.vector.memset(go[:, 0:h, :], 0.0)
                        nc.scalar.memzero(go[:, h:max_no, :])
                for _, par, c0, c1, ridx in calls_by_seg.get(s, []):
                    gt = ge if par == 0 else go
                    tbl = xe_d if par == 0 else xo_d
                    isb = idxe_sb if par == 0 else idxo_sb
                    goff = int(egoff[s]) if par == 0 else int(ogoff[s])
                    if ridx >= 0:
                        reg = gregs[rn % 2]
                        rn += 1
                        nc.reg_load(reg, cnt_sb[0:1, ridx : ridx + 1])
                        nreg = reg
                    else:
                        nreg = (c1 - c0) * P
                    nc.gpsimd.dma_gather(
                        gt[:, c0:c1, :],
                        tbl[:, :],
                        isb[:, 8 * (goff + c0) : 8 * (goff + c1)],
                        (c1 - c0) * P,
                        nreg,
                        WIDTH,
                        queue_num=QORD[qn % 4],
                    )
                    qn += 1
                # S loads are hoisted two segments ahead so they enter the
                # Sync queue before the output stores that wait on compute
                # (head-of-line blocking would starve the PE mid-run)
                if s == 0:
                    for sp in range(min(3, n_segs)):
                        _load_sc(sp)
                elif s + 2 < n_segs:
                    _load_sc(s + 2)
                sc, st0 = sc_tiles.pop(s)
                for bb in blocks:
                    seq = [(ge, tt) for tt in range(lo_e[bb], hi_e[bb])] + [
                        (go, tt) for tt in range(lo_o[bb], hi_o[bb])
                    ]
                    ntb = len(seq)
                    assert ntb > 0
                    ps0 = ppool.tile([P, P], f32, tag="ps0")
                    ps1 = ppool.tile([P, P], f32, tag="ps1")
                    for k, (gt, col) in enumerate(seq):
                        s_ap = sc[:, (int(soff[bb]) - st0 + k) * P : (int(soff[bb]) - st0 + k + 1) * P]
                        nc.tensor.matmul(
                            out=ps0[:],
                            lhsT=gt[:, col, 0:P],
                            rhs=s_ap,
                            start=(k == 0),
                            stop=(k == ntb - 1),
                        )
                        nc.tensor.matmul(
                            out=ps1[:],
                            lhsT=gt[:, col, P:WIDTH],
                            rhs=s_ap,
                            start=(k == 0),
                            stop=(k == ntb - 1),
                        )
                    # PSUM -> SBUF cast; the self-loop term (dinv*x_self)^T
                    # rides along as a free tensor_tensor add
                    a = apool.tile([P, 2 * P], bf16, tag="a")
                    nc.vector.tensor_tensor(
                        out=a[:, 0:P],
                        in0=ps0[:],
                        in1=xself_sb[:, bb * W2 : bb * W2 + P],
                        op=Alu.add,
                    )
                    nc.vector.tensor_tensor(
                        out=a[:, P : 2 * P],
                        in0=ps1[:],
                        in1=xself_sb[:, bb * W2 + P : (bb + 1) * W2],
                        op=Alu.add,
                    )
                    po = opsum.tile([P, NW], f32, tag="po")
                    nc.tensor.matmul(
                        out=po[:],
                        lhsT=a[:, 0:P],
                        rhs=wt_sb[:, 0:NW],
                        start=True,
                        stop=False,
                    )
                    nc.tensor.matmul(
                        out=po[:],
                        lhsT=a[:, P : 2 * P],
                        rhs=wt_sb[:, NW : 2 * NW],
                        start=False,
                        stop=False,
                    )
                    nc.tensor.matmul(
                        out=po[:],
                        lhsT=vrow_sb[:, NW + bb * P : NW + (bb + 1) * P],
                        rhs=vrow_sb[:, 0:NW],
                        start=False,
                        stop=True,
                    )
                    sq = sqpool.tile([P, WIDTH], f32, tag="sq")
                    ssq = stat.tile([P, 1], f32, tag="ssq")
                    nc.scalar.activation(
                        out=sq[:], in_=po[:, :WIDTH], func=Act.Square, accum_out=ssq[:]
                    )
                    m2 = stat.tile([P, 1], f32, tag="m2")
                    nc.scalar.activation(
                        out=m2[:],
                        in_=po[:, WIDTH : WIDTH + 1],
                        func=Act.Square,
                        scale=1.0 / WIDTH,
                    )
                    rv = stat.tile([P, 1], f32, tag="rv")
                    nc.vector.tensor_scalar(
                        out=rv[:],
                        in0=ssq[:],
                        scalar1=1.0 / WIDTH,
                        scalar2=m2[:, :1],
                        op0=Alu.mult,
                        op1=Alu.subtract,
                    )
                    sd = stat.tile([P, 1], f32, tag="sd")
                    nc.scalar.activation(
                        out=sd[:], in_=rv[:], func=Act.Sqrt, bias=fcon_sb[:, bb : bb + 1]
                    )
                    rstd = stat.tile([P, 1], f32, tag="rstd")
                    nc.vector.reciprocal(rstd[:], sd[:])
                    nb = stat.tile([P, 1], f32, tag="nb")
                    nc.vector.tensor_scalar(
                        out=nb[:],
                        in0=po[:, WIDTH : WIDTH + 1],
                        scalar1=-1.0 / WIDTH,
                        scalar2=rstd[:, :1],
                        op0=Alu.mult,
                        op1=Alu.mult,
                    )
                    yo = ypool.tile([P, WIDTH], f32, tag="yo")
                    if not generic_affine:
                        nc.scalar.activation(
                            out=yo[:],
                            in_=po[:, :WIDTH],
                            func=Act.Relu,
                            scale=rstd[:, :1],
                            bias=nb[:, :1],
                        )
                    else:
                        t1 = ypool.tile([P, WIDTH], f32, tag="t1")
                        nc.scalar.activation(
                            out=t1[:],
                            in_=po[:, :WIDTH],
                            func=Act.Identity,
                            scale=rstd[:, :1],
                            bias=nb[:, :1],
                        )
                        t2 = ypool.tile([P, WIDTH], f32, tag="t2")
                        nc.vector.tensor_tensor(
                            out=t2[:], in0=t1[:], in1=gb_sb[:, :WIDTH], op=Alu.mult
                        )
                        t3 = ypool.tile([P, WIDTH], f32, tag="t3")
                        nc.vector.tensor_tensor(
                            out=t3[:], in0=t2[:], in1=gb_sb[:, WIDTH:], op=Alu.add
                        )
                        nc.scalar.activation(out=yo[:], in_=t3[:], func=Act.Relu)
                    rows = min(P, NODES_PER_CORE - bb * P)
                    nc.sync.dma_start(out_d[bb * P : bb * P + rows, :], yo[:rows, :])
    return nc


def _pack_inputs(meta, deg, idxe, idxo, scon, cnts, x, W, bias, gamma, beta, generic_affine):
    bfnp = ml_dtypes.bfloat16

    dinv = (1.0 / np.sqrt(deg)).astype(np.float64)
    xs = (np.asarray(x, np.float64) * dinv[:, None]).astype(bfnp)
    xe = np.ascontiguousarray(xs[0::2])
    xo = np.ascontiguousarray(xs[1::2])

    # transposed self tiles: xself[c][p, b*256 + h*128 + d] = xs_pad[b*128+d, h*128+p]
    xself_all = np.zeros((N_CORES, P, N_BLOCKS * WIDTH), bfnp)
    for c in range(N_CORES):
        sl = xs[c * NODES_PER_CORE : (c + 1) * NODES_PER_CORE]
        flat = np.zeros((N_BLOCKS * P, WIDTH), bfnp)
        flat[: NODES_PER_CORE] = sl
        xself_all[c] = np.ascontiguousarray(
            flat.reshape(N_BLOCKS, P, 2, P)
            .transpose(3, 0, 2, 1)
            .reshape(P, N_BLOCKS * WIDTH)
        )

    degp = np.ones((N_CORES, N_BLOCKS * P), np.float64)
    for c in range(N_CORES):
        degp[c, :NODES_PER_CORE] = deg[c * NODES_PER_CORE : (c + 1) * NODES_PER_CORE]
    epsdeg = (LN_EPS * degp).astype(np.float32).reshape(N_CORES, N_BLOCKS, P)
    epsdeg = np.ascontiguousarray(epsdeg.transpose(0, 2, 1))

    WT32 = np.asarray(W, np.float64).T
    rs = WT32.sum(axis=1, keepdims=True)
    WTe = np.concatenate([WT32, rs], axis=1).astype(bfnp)
    wt = np.concatenate([WTe[:P], WTe[P:]], axis=1)
    ident = np.eye(P, dtype=bfnp)
    wcon = np.ascontiguousarray(np.concatenate([wt, ident], axis=1))

    b64 = np.asarray(bias, np.float64)
    brow = np.concatenate([b64, [b64.sum()]])
    sdeg = np.sqrt(degp)
    vrow_all = np.concatenate(
        [np.tile(brow[None, :], (N_CORES, 1)), sdeg], axis=1
    ).astype(bfnp)

    in_maps = []
    for c in range(N_CORES):
        m = {
            "xe": xe,
            "xo": xo,
            "idxe": np.ascontiguousarray(idxe[c]),
            "idxo": np.ascontiguousarray(idxo[c]),
            "scon": np.ascontiguousarray(scon[c]),
            "xself": xself_all[c],
            "fcon": epsdeg[c],
            "wcon": wcon,
            "vrow": vrow_all[c : c + 1],
            "cnts": np.ascontiguousarray(cnts[c]),
        }
        if generic_affine:
            gb = np.concatenate(
                [
                    np.tile(np.asarray(gamma, np.float32)[None, :], (P, 1)),
                    np.tile(np.asarray(beta, np.float32)[None, :], (P, 1)),
                ],
                axis=1,
            )
            m["gbcon"] = np.ascontiguousarray(gb)
        in_maps.append(m)
    return in_maps


_PROGRAM_CACHE = {}


def kernel(x, edge_index, W, b, gamma, beta, _run_kwargs=None):
    from concourse.bass_utils import run_bass_kernel_spmd

    x = np.asarray(x)
    W = np.asarray(W)
    bias = np.asarray(b)
    gamma = np.asarray(gamma)
    beta = np.asarray(beta)

    meta, deg, idxe, idxo, scon, cnts = _preprocess(edge_index)
    generic_affine = not (np.all(gamma == 1.0) and np.all(beta == 0.0))

    key = (meta, generic_affine)
    if key not in _PROGRAM_CACHE:
        nc = _build_program(meta, generic_affine)
        nc.finalize()
        _PROGRAM_CACHE[key] = nc
    nc = _PROGRAM_CACHE[key]

    in_maps = _pack_inputs(
        meta, deg, idxe, idxo, scon, cnts, x, W, bias, gamma, beta, generic_affine
    )

    kwargs = dict(_run_kwargs or {})
    kwargs.pop("_result", None)
    rr = run_bass_kernel_spmd(nc, in_maps, list(range(N_CORES)), **kwargs)
    out = np.concatenate([rr.results[c]["out"] for c in range(N_CORES)], axis=0)
    if _run_kwargs is not None:
        _run_kwargs["_result"] = rr
    return np.ascontiguousarray(out.astype(np.float32))
